# revision 11
# baseline (speedup 1.0000x reference)
"""GQA attention (16 q heads / 4 kv heads, HD=128, S=4096, D=2048) with RoPE,
causal mask, log-gate on kv positions, softmax, and output projection —
distributed over 8 NeuronCores.

Sharding: head-parallel. Core c computes q heads {2c, 2c+1} and kv head c//2.
Wq/Wk/Wv are split column-wise, Wo row-wise; each core produces a partial
[S, D] output (its 2 heads' contribution through Wo) and the host sums the 8
partials (the unshard step of the row-parallel Wo matmul).

On-device layout strategy (v2):
 - All matmul inputs fp16 (1 cycle/row like bf16, 8x the mantissa).
 - Projections computed transposed: qT/kT = W.T @ X.T with d on partitions,
   which is exactly the layout flash-attention needs.
 - Attention computed transposed (scores^T [j, i]); max-free softmax with a
   constant -8 shift so fp16 exp/accumulate never overflows.
 - log-gate + shift folded into the exp as a per-partition activation bias
   (keys are partitions in the transposed layout).
 - Causality applied structurally: upper-triangle blocks skipped, diagonal
   blocks masked by a 0/1 fp16 multiply AFTER exp (cheap 16-bit DVE op).
 - Softmax denominators: fp16 DVE accumulation of exp tiles across key
   blocks, then one M=1 ones-matmul per head (kills the per-block M=1
   matmuls that burned ~90us of PE time), reciprocal_approx_fast, gpsimd
   partition-broadcast of the reciprocal.
 - Emission pipelined: attn(nb) -> sums -> proj(nb+1) -> normalize(nb) ->
   outproj(nb), so the normalize chain hides under proj matmuls.
 - Per-core partial outputs written fp16 (halves write traffic; host sums
   in fp32).
"""

import math
from contextlib import ExitStack

import numpy as np

import concourse.bass as bass
import concourse.mybir as mybir
import concourse.tile as tile
from concourse import bacc
from concourse._compat import with_exitstack
from concourse.bass import ds
from concourse.bass_utils import run_bass_kernel_spmd
from concourse.masks import make_identity

P = 128
F = 512            # free-dim chunk (one PSUM bank of fp32)
S = 4096
D = 2048
HD = 128
KO = D // P        # 16 k-chunks for the projections
NB = S // F        # 8 sequence chunks
NJB = S // P       # 32 key blocks
F32 = mybir.dt.float32
FP16 = mybir.dt.float16
SHIFT = 8.0        # constant softmax shift (cancels in the ratio)


@with_exitstack
def _body(ctx: ExitStack, tc: tile.TileContext, io: dict):
    nc = tc.nc

    persist = ctx.enter_context(tc.tile_pool(name="persist", bufs=1))
    qT = persist.tile([P, 2, S], FP16, tag="qT")        # [d, h, i]
    kT = persist.tile([P, S], FP16, tag="kT")           # [d, j]
    vv = persist.tile([P, NJB, HD], FP16, tag="vv")     # [j, jb, d]
    attnT = persist.tile([P, 2, S], FP16, tag="attnT")  # [d, h, i] normalized
    logg = persist.tile([P, NJB], F32, tag="logg")      # log(gate)-SHIFT, [j, jb]
    dmask01 = persist.tile([P, P], FP16, tag="dmask01")
    ident = persist.tile([P, P], F32, tag="ident")
    ones16 = persist.tile([P, 1], FP16, tag="ones16")

    wpool = ctx.enter_context(tc.tile_pool(name="wpool", bufs=1))
    wq = wpool.tile([P, KO, 2 * HD], FP16, tag="wq")
    wk = wpool.tile([P, KO, HD], FP16, tag="wk")
    wv = wpool.tile([P, KO, HD], FP16, tag="wv")
    wo = wpool.tile([P, 2, D], FP16, tag="wo")

    xt_r = io["xt"].rearrange("(ko p) s -> p ko s", p=P)  # [128, 16, 4096]

    xt_pool = ctx.enter_context(tc.tile_pool(name="xt", bufs=10))
    tab_pool = ctx.enter_context(tc.tile_pool(name="tab", bufs=2))
    rope_pool = ctx.enter_context(tc.tile_pool(name="rope", bufs=2))
    exp_pool = ctx.enter_context(tc.tile_pool(name="exp", bufs=4))
    acc_pool = ctx.enter_context(tc.tile_pool(name="acc", bufs=2))
    bc_pool = ctx.enter_context(tc.tile_pool(name="bc", bufs=2))
    ob_pool = ctx.enter_context(tc.tile_pool(name="ob", bufs=2))
    # PSUM budget (8 banks): psSc pair tiles 2x2 + psAV 3 + psSum 1 = 8.
    psSc = ctx.enter_context(tc.tile_pool(name="psSc", bufs=2, space="PSUM"))
    psAV = ctx.enter_context(tc.tile_pool(name="psAV", bufs=3, space="PSUM"))
    psSum = ctx.enter_context(tc.tile_pool(name="psSum", bufs=1, space="PSUM"))

    def load_x(nb):
        sl = ds(nb * F, F)
        xq = []
        for xi in range(4):
            xtile = xt_pool.tile([P, 4, F], FP16, tag="xt")
            nc.sync.dma_start(xtile[:], xt_r[:, ds(xi * 4, 4), sl])
            xq.append(xtile)
        tabs = tab_pool.tile([P, 4, F], F32, tag="tabs")
        nc.sync.dma_start(tabs[:], io["tabs"][:, :, sl])
        return xq, tabs

    # Startup: x-tiles + q-weights first so the first proj chain can start
    # while the remaining weights/constants stream in.
    xq0, tabs0 = load_x(0)
    wq_r = io["wq"].rearrange("(ko p) m -> p ko m", p=P)
    for ko in range(KO):
        nc.sync.dma_start(wq[:, ko, :], wq_r[:, ko, :])
    nc.sync.dma_start(wk[:], io["wk"].rearrange("(ko p) m -> p ko m", p=P))
    nc.sync.dma_start(wv[:], io["wv"].rearrange("(ko p) m -> p ko m", p=P))
    nc.sync.dma_start(logg[:], io["logg"])
    nc.sync.dma_start(dmask01[:], io["dmask01"])
    nc.sync.dma_start(wo[:], io["wo"].rearrange("(h p) e -> p h e", p=P))
    make_identity(nc, ident[:])
    nc.vector.memset(ones16[:], 1.0)

    def proj(nb, xq, tabs):
        """Projections + rope + v-transpose for sequence chunk nb."""
        sl = ds(nb * F, F)

        def chain(w_sb, m0):
            ps = psSc.tile([P, 2, F], F32, tag="sc")
            for ko in range(KO):
                nc.tensor.matmul(
                    ps[:, 0, :],
                    lhsT=w_sb[:, ko, ds(m0, P)],
                    rhs=xq[ko // 4][:, ko % 4, :],
                    start=(ko == 0),
                    stop=(ko == KO - 1),
                )
            return ps

        def rope(ps, ct, st, dest):
            tmp = rope_pool.tile([P, F], F32, tag="tmp")
            nc.scalar.copy(tmp[:], ps[:, 0, :])
            rot = rope_pool.tile([P, F], F32, tag="rot")
            nc.sync.dma_start(rot[0:64, :], tmp[64:128, :])
            nc.sync.dma_start(rot[64:128, :], tmp[0:64, :])
            r2 = rope_pool.tile([P, F], F32, tag="r2")
            nc.vector.tensor_tensor(r2[:], rot[:], st, op=mybir.AluOpType.mult)
            t1 = rope_pool.tile([P, F], F32, tag="t1")
            nc.vector.tensor_tensor(t1[:], tmp[:], ct, op=mybir.AluOpType.mult)
            nc.vector.tensor_tensor(dest, t1[:], r2[:], op=mybir.AluOpType.add)

        rope(chain(wq, 0), tabs[:, 0, :], tabs[:, 1, :], qT[:, 0, sl])
        rope(chain(wq, P), tabs[:, 0, :], tabs[:, 1, :], qT[:, 1, sl])
        rope(chain(wk, 0), tabs[:, 2, :], tabs[:, 3, :], kT[:, sl])

        psv = chain(wv, 0)
        vT = rope_pool.tile([P, F], F32, tag="vT")
        nc.scalar.copy(vT[:], psv[:, 0, :])
        pt = psSc.tile([P, 2, F], F32, tag="sc")
        for isub in range(4):
            nc.tensor.transpose(
                pt[:, 0, ds(isub * P, P)], vT[:, ds(isub * P, P)], ident[:]
            )
        nc.scalar.copy(vv[:, ds(nb * 4, 4), :], pt[:, 0, :])

    def attn(nb):
        """Attention for q-chunk nb (both heads); returns psum AV tiles +
        the fp16 exp-accumulator (for the sum collapse)."""
        sl = ds(nb * F, F)
        njb = 4 * nb + 4
        avs = [
            psAV.tile([P, F], F32, tag="av", name=f"av{h}") for h in range(2)
        ]
        acc = acc_pool.tile([P, 2, F], FP16, tag="acc")
        for jb in range(njb):
            dp = jb - 4 * nb
            # Queries below the diagonal block are fully masked: skip them.
            q0 = dp * P if dp > 0 else 0
            n = F - q0
            sc = psSc.tile([P, 2, F], F32, tag="sc")
            for h in range(2):
                nc.tensor.matmul(
                    sc[:, h, 0:n],
                    lhsT=kT[:, ds(jb * P, P)],
                    rhs=qT[:, h, ds(nb * F + q0, n)],
                    start=True,
                    stop=True,
                )
            ex = exp_pool.tile([P, 2, F], FP16, tag="ex")
            nc.scalar.activation(
                ex[:, :, 0:n], sc[:, :, 0:n],
                mybir.ActivationFunctionType.Exp,
                bias=logg[:, jb : jb + 1],
            )
            if dp >= 0:
                # within-block triangle mask on the diagonal 128 queries
                for h in range(2):
                    nc.vector.tensor_tensor(
                        ex[:, h, 0:P], ex[:, h, 0:P], dmask01[:],
                        op=mybir.AluOpType.mult,
                    )
            if jb == 0:
                nc.vector.tensor_copy(acc[:], ex[:])
            else:
                nc.vector.tensor_tensor(
                    acc[:, :, q0:F], acc[:, :, q0:F], ex[:, :, 0:n],
                    op=mybir.AluOpType.add,
                )
            for h in range(2):
                nc.tensor.matmul(
                    avs[h][:, q0:F],
                    lhsT=vv[:, jb, :],
                    rhs=ex[:, h, 0:n],
                    start=(jb == 0),
                    stop=(jb == njb - 1),
                )
        sums = psSum.tile([64, F], F32, tag="sums")
        for h in range(2):
            nc.tensor.matmul(
                sums[ds(32 * h, 1), :],
                lhsT=ones16[:, 0:1],
                rhs=acc[:, h, :],
                start=True,
                stop=True,
            )
        return avs, sums

    def normalize(nb, avs, sums):
        sl = ds(nb * F, F)
        for h in range(2):
            srow = bc_pool.tile([1, F], F32, tag=f"srow{h}", name=f"srow{h}")
            nc.scalar.copy(srow[:], sums[ds(32 * h, 1), :])
            rrow = bc_pool.tile([1, F], F32, tag=f"rrow{h}", name=f"rrow{h}")
            nc.vector.reciprocal_approx_fast(rrow[:], srow[:])
            rbc = bc_pool.tile([P, F], F32, tag=f"rbc{h}")
            nc.gpsimd.partition_broadcast(rbc[:], rrow[0:1, :])
            nc.vector.tensor_tensor(
                attnT[:, h, sl], avs[h][:], rbc[:],
                op=mybir.AluOpType.mult,
            )

    def outproj(nb):
        for i4 in range(4):
            i2 = nb * 4 + i4
            ob = ob_pool.tile([P, D], FP16, tag="ob")
            for e in range(D // F):
                po = psAV.tile([P, F], F32, tag="av")
                for h in range(2):
                    nc.tensor.matmul(
                        po[:],
                        lhsT=attnT[:, h, ds(i2 * P, P)],
                        rhs=wo[:, h, ds(e * F, F)],
                        start=(h == 0),
                        stop=(h == 1),
                    )
                if e % 2 == 0:
                    nc.scalar.copy(ob[:, ds(e * F, F)], po[:])
                else:
                    nc.vector.tensor_copy(ob[:, ds(e * F, F)], po[:])
            nc.sync.dma_start(
                io["outp"][ds(i2 * P, P), 0 : D // 2], ob[:, 0 : D // 2]
            )
            nc.sync.dma_start(
                io["outp"][ds(i2 * P, P), D // 2 : D], ob[:, D // 2 : D]
            )

    # ---- pipelined emission ----
    # load_x(nb+1) is issued right after proj(nb) so its DMAs are queued
    # ahead of the outproj(nb-1) output burst and land well before
    # proj(nb+1) needs them.
    proj(0, xq0, tabs0)
    xqn, tabsn = load_x(1)
    for nb in range(NB):
        avs, sums = attn(nb)
        if nb + 1 < NB:
            xq_cur, tabs_cur = xqn, tabsn
            proj(nb + 1, xq_cur, tabs_cur)
            if nb + 2 < NB:
                xqn, tabsn = load_x(nb + 2)
        normalize(nb, avs, sums)
        outproj(nb)


_NC_CACHE = None


def build_nc():
    global _NC_CACHE
    if _NC_CACHE is not None:
        return _NC_CACHE
    nc = bacc.Bacc("TRN2", target_bir_lowering=False, debug=False)
    io = {
        "xt": nc.dram_tensor("xt", [D, S], FP16, kind="ExternalInput").ap(),
        "wq": nc.dram_tensor("wq", [D, 2 * HD], FP16, kind="ExternalInput").ap(),
        "wk": nc.dram_tensor("wk", [D, HD], FP16, kind="ExternalInput").ap(),
        "wv": nc.dram_tensor("wv", [D, HD], FP16, kind="ExternalInput").ap(),
        "wo": nc.dram_tensor("wo", [2 * HD, D], FP16, kind="ExternalInput").ap(),
        "tabs": nc.dram_tensor("tabs", [P, 4, S], F32, kind="ExternalInput").ap(),
        "logg": nc.dram_tensor("logg", [P, NJB], F32, kind="ExternalInput").ap(),
        "dmask01": nc.dram_tensor(
            "dmask01", [P, P], FP16, kind="ExternalInput"
        ).ap(),
        "outp": nc.dram_tensor("outp", [S, D], FP16, kind="ExternalOutput").ap(),
    }
    with tile.TileContext(nc) as tc:
        _body(tc, io)
    nc.compile()
    _NC_CACHE = nc
    return nc


def make_in_maps(hidden_states, attention_mask, cos, sin, gate, Wq, Wk, Wv, Wo):
    X = np.asarray(hidden_states, np.float32).reshape(S, D)
    xt = np.ascontiguousarray(X.T.astype(np.float16))
    cosT = np.ascontiguousarray(np.asarray(cos, np.float32).reshape(S, HD).T)
    sinT = np.ascontiguousarray(np.asarray(sin, np.float32).reshape(S, HD).T)
    sinTs = np.concatenate([-sinT[: HD // 2], sinT[HD // 2 :]], axis=0)
    sc = np.float32(1.0 / math.sqrt(HD))
    tabs = np.ascontiguousarray(
        np.stack([cosT * sc, sinTs * sc, cosT, sinTs], axis=1)
    )
    g = np.asarray(gate, np.float32).reshape(S) + np.float32(1e-8)
    logg = np.log(g).astype(np.float32) - np.float32(SHIFT)
    logg = np.ascontiguousarray(logg.reshape(NJB, P).T)
    jj = np.arange(P)[:, None]
    ii = np.arange(P)[None, :]
    dmask01 = np.ascontiguousarray(
        np.where(jj <= ii, 1, 0).astype(np.float16)
    )

    Wq = np.asarray(Wq, np.float32)
    Wk = np.asarray(Wk, np.float32)
    Wv = np.asarray(Wv, np.float32)
    Wo = np.asarray(Wo, np.float32)

    in_maps = []
    for c in range(8):
        g128 = c // 2
        in_maps.append(
            {
                "xt": xt,
                "wq": np.ascontiguousarray(
                    Wq[:, c * 256 : (c + 1) * 256].astype(np.float16)
                ),
                "wk": np.ascontiguousarray(
                    Wk[:, g128 * HD : (g128 + 1) * HD].astype(np.float16)
                ),
                "wv": np.ascontiguousarray(
                    Wv[:, g128 * HD : (g128 + 1) * HD].astype(np.float16)
                ),
                "wo": np.ascontiguousarray(
                    Wo[c * 256 : (c + 1) * 256, :].astype(np.float16)
                ),
                "tabs": tabs,
                "logg": logg,
                "dmask01": dmask01,
            }
        )
    return in_maps


def kernel(hidden_states, attention_mask, cos, sin, gate, Wq, Wk, Wv, Wo,
           **kwargs):
    nc = build_nc()
    in_maps = make_in_maps(
        hidden_states, attention_mask, cos, sin, gate, Wq, Wk, Wv, Wo
    )
    res = run_bass_kernel_spmd(nc, in_maps, core_ids=list(range(8)), **kwargs)
    acc = res.results[0]["outp"].astype(np.float32)
    for c in range(1, 8):
        acc = acc + res.results[c]["outp"].astype(np.float32)
    out = acc.reshape(1, S, D)
    if kwargs:
        return out, res
    return out
